# revision 1
# baseline (speedup 1.0000x reference)
"""Sparse 3D conv (gather -> per-offset matmul -> scatter-add) on 8 Trainium2
NeuronCores.

Strategy (data-parallel over rulebook rows, partitioned by output voxel):
  - Each core owns an equal slice of output rows (N/8 = 12500) + trash rows.
  - Host routes each rulebook entry (k, m) to the core owning out_idx[k,m].
  - Within a core, entries are grouped (k, input-chunk ch) for int16 gather
    indices, and within (k, ch) split by class ci = within-(core,k) bin rank.
  - dma_gather (non-transpose) fetches interleaved bf16 hi/lo rows [hi|lo]
    (256B) row-major; PE transposes each 128-chunk, then computes
    Y = (Xh+Xl) @ (Wh+Wl) via two bf16 matmuls accumulating in PSUM
    (split-bf16 ~1e-5 rel err). Y chunks land in a per-k staging tile in
    class-major slot order.
  - dma_scatter_add (CCE DMA add) pushes each (k, class) slice into the core's
    HBM output slice. Within one instruction all bins are unique (class =
    within-k bin rank), so duplicate-index RMW races cannot occur; across
    instructions Tile's WAW dependency on the output tensor serializes
    completion. Padding entries land in trash rows, dropped on unshard.
"""
import sys

if "/opt/trn_rl_repo" not in sys.path:
    sys.path.insert(0, "/opt/trn_rl_repo")

import numpy as np
import ml_dtypes

from concourse import tile, bacc
from concourse import mybir
from concourse.bass_utils import run_bass_kernel_spmd
from concourse.library_config import mlp

BF16 = ml_dtypes.bfloat16

NCORES = 8
CHK = 25000          # gather source chunk rows (int16 index limit)
TRASH = 12           # trash rows appended to each core's output slice
SIM = False          # run in CoreSim (MultiCoreSim) instead of hardware
ABLATE = None        # None | "no_scatter" | "no_compute" (perf attribution)
BUILD_ONLY = False   # build+compile only; stash nc/in_maps and return zeros

LAST_RESULTS = None  # BassKernelResults of the most recent run (profiling)
LAST_NC = None
LAST_IN_MAPS = None


def _wrap_idx_cols(idx16):
    """[G] int16 -> [128, G//16] wrapped (j -> [j%16, j//16]) replicated x8."""
    n = idx16.shape[0]
    w = np.asarray(idx16, np.int16).reshape(n // 16, 16).T
    return np.tile(w, (8, 1))


def _build_program(K, C, nchk, plan, rows_out, ncols16):
    nc = bacc.Bacc(None, target_bir_lowering=False, debug=False,
                   num_swdge_queues=1)

    feats_d = nc.dram_tensor("feats_hl", [nchk * CHK, 2 * C], mybir.dt.bfloat16,
                             kind="ExternalInput")
    gidx_d = nc.dram_tensor("gidx", [128, ncols16], mybir.dt.int16,
                            kind="ExternalInput")
    sidx_d = nc.dram_tensor("sidx", [128, ncols16], mybir.dt.int16,
                            kind="ExternalInput")
    whh_d = nc.dram_tensor("whh", [2 * C, K * C], mybir.dt.bfloat16,
                           kind="ExternalInput")
    wll_d = nc.dram_tensor("wll", [2 * C, K * C], mybir.dt.bfloat16,
                           kind="ExternalInput")
    ident_d = nc.dram_tensor("ident", [128, 128], mybir.dt.bfloat16,
                             kind="ExternalInput")
    out_d = nc.dram_tensor("out", [rows_out, C], mybir.dt.float32,
                           kind="ExternalOutput")

    max_slots = max((p["nslots"] for p in plan), default=1)

    with tile.TileContext(nc) as tc:
        with (
            tc.tile_pool(name="const", bufs=1) as cpool,
            tc.tile_pool(name="xt", bufs=3) as xt_pool,
            tc.tile_pool(name="xtc", bufs=8) as xtc_pool,
            tc.tile_pool(name="yk", bufs=2) as y_pool,
            tc.tile_pool(name="pst", bufs=4, space="PSUM") as pst_pool,
            tc.tile_pool(name="psy", bufs=4, space="PSUM") as psy_pool,
        ):
            nc.gpsimd.load_library(mlp)
            gidx_t = cpool.tile([128, ncols16], mybir.dt.int16)
            sidx_t = cpool.tile([128, ncols16], mybir.dt.int16)
            whh_t = cpool.tile([2 * C, K * C], mybir.dt.bfloat16)
            wll_t = cpool.tile([2 * C, K * C], mybir.dt.bfloat16)
            ident_t = cpool.tile([128, 128], mybir.dt.bfloat16)
            nc.sync.dma_start(gidx_t[:], gidx_d[:])
            nc.sync.dma_start(sidx_t[:], sidx_d[:])
            nc.sync.dma_start(whh_t[:], whh_d[:])
            nc.sync.dma_start(wll_t[:], wll_d[:])
            nc.sync.dma_start(ident_t[:], ident_d[:])

            zero_t = cpool.tile([128, C], mybir.dt.float32)
            nc.vector.memset(zero_t[:], 0.0)
            for i in range(0, rows_out, 128):
                h = min(128, rows_out - i)
                nc.sync.dma_start(out_d[i:i + h, :], zero_t[:h, :])

            nch_done = 0
            for k in range(K):
                p = plan[k]
                if p["nslots"] == 0:
                    continue
                y_k = y_pool.tile([128, max_slots, C], mybir.dt.float32,
                                  tag="yk")
                for (ch, m_all, goff_all, slots_all) in p["gathers"]:
                  for sub in range(0, m_all, 8):
                    m = min(8, m_all - sub)
                    goff = goff_all + sub * 8
                    slots = slots_all[sub:sub + m]
                    G = m * 128
                    xt = xt_pool.tile([128, m, 2 * C], mybir.dt.bfloat16,
                                      tag="xt")
                    nc.gpsimd.dma_gather(
                        xt[:],
                        feats_d[ch * CHK:(ch + 1) * CHK, :],
                        gidx_t[:, goff:goff + G // 16],
                        G, G, 2 * C, queue_num=0,
                    )
                    for j in range(m if ABLATE != "no_compute" else 0):
                        t_ps = pst_pool.tile([128, 128], mybir.dt.bfloat16,
                                             tag="pt")
                        nc.tensor.transpose(t_ps[:], xt[:, j, :], ident_t[:])
                        xt_col = xtc_pool.tile([128, 128], mybir.dt.bfloat16,
                                               tag="xtc")
                        if nch_done % 2 == 0:
                            nc.vector.tensor_copy(xt_col[:], t_ps[:])
                        else:
                            nc.scalar.copy(xt_col[:], t_ps[:])
                        y_ps = psy_pool.tile([128, C], mybir.dt.float32,
                                             tag="py")
                        nc.tensor.matmul(y_ps[:], xt_col[:],
                                         whh_t[:, k * C:(k + 1) * C],
                                         start=True, stop=False)
                        nc.tensor.matmul(y_ps[:], xt_col[:],
                                         wll_t[:, k * C:(k + 1) * C],
                                         start=False, stop=True)
                        s = slots[j]
                        if nch_done % 2 == 0:
                            nc.scalar.copy(y_k[:, s, :], y_ps[:])
                        else:
                            nc.vector.tensor_copy(y_k[:, s, :], y_ps[:])
                        nch_done += 1
                for (lo_all, hi_all, soff_all) in (p["scatters"] if ABLATE != "no_scatter" else []):
                  for lo in range(lo_all, hi_all, 8):
                    hi = min(lo + 8, hi_all)
                    soff = soff_all + (lo - lo_all) * 8
                    G = (hi - lo) * 128
                    nc.gpsimd.dma_scatter_add(
                        out_d[:], y_k[:, lo:hi, :],
                        sidx_t[:, soff:soff + G // 16],
                        G, G, C, queue_num=0,
                    )

    nc.compile()
    return nc


def _route(in_idx, out_idx, rows_per_core, K, nchk):
    """Per (core,k,ch,class) entry index lists; class = within-(core,k) bin
    rank."""
    sel_all = {}
    maxcls = 1
    for k in range(K):
        core_k = out_idx[k] // rows_per_core
        ch_k = in_idx[k] // CHK
        for c in range(NCORES):
            m = np.nonzero(core_k == c)[0]
            if len(m) == 0:
                continue
            bins = out_idx[k][m] - c * rows_per_core
            order = np.argsort(bins, kind="stable")
            sb = bins[order]
            grp_start = np.zeros(len(sb), np.int64)
            if len(sb) > 1:
                new_grp = np.nonzero(sb[1:] != sb[:-1])[0] + 1
                starts = np.zeros(len(sb), np.int64)
                starts[new_grp] = new_grp
                grp_start = np.maximum.accumulate(starts)
            rank_sorted = np.arange(len(sb)) - grp_start
            rank = np.empty(len(m), np.int64)
            rank[order] = rank_sorted
            maxcls = max(maxcls, int(rank.max()) + 1)
            chv = ch_k[m]
            for ch in range(nchk):
                for ci in range(int(rank.max()) + 1):
                    sel = m[(chv == ch) & (rank == ci)]
                    if len(sel):
                        sel_all[(c, k, ch, ci)] = sel
    return sel_all, maxcls


def kernel(feats, kernel, in_idx, out_idx):
    feats = np.asarray(feats, np.float32)
    Wk = np.asarray(kernel, np.float32)
    in_idx = np.asarray(in_idx, np.int64)
    out_idx = np.asarray(out_idx, np.int64)

    N, C = feats.shape
    K, M = in_idx.shape
    rows_per_core = (N + NCORES - 1) // NCORES
    rows_out = rows_per_core + TRASH
    nchk = (N + CHK - 1) // CHK

    # ---- precision split + weight stacking ----
    f_hi = feats.astype(BF16)
    f_lo = (feats - f_hi.astype(np.float32)).astype(BF16)
    feats_hl = np.zeros((nchk * CHK, 2 * C), BF16)
    feats_hl[:N, :C] = f_hi
    feats_hl[:N, C:] = f_lo

    W_hi = Wk.astype(BF16)
    W_lo = (Wk - W_hi.astype(np.float32)).astype(BF16)
    whh = np.ascontiguousarray(
        np.concatenate([W_hi, W_hi], axis=1).transpose(1, 0, 2).reshape(2 * C, K * C))
    wll = np.ascontiguousarray(
        np.concatenate([W_lo, W_lo], axis=1).transpose(1, 0, 2).reshape(2 * C, K * C))
    ident = np.eye(128, dtype=BF16)

    sel_all, maxcls = _route(in_idx, out_idx, rows_per_core, K, nchk)

    # static caps per (k, ch, ci) = roundup128(max over cores)
    cap = {}
    for (c, k, ch, ci), sel in sel_all.items():
        key = (k, ch, ci)
        cap[key] = max(cap.get(key, 0), len(sel))
    cap = {key: (v + 127) // 128 * 128 for key, v in cap.items()}

    # ---- shared plan + segment table ----
    plan = []
    col = 0          # shared column cursor (16-entry units) for both planes
    segs = {}        # (k, ch, ci) -> (gcol, scol, G)
    for k in range(K):
        gathers = []
        scatters = []
        slot = 0
        slot_of = {}
        # scatter layout: class-major, ch-minor
        scol_of = {}
        for ci in range(maxcls):
            lo = slot
            soff = col
            for ch in range(nchk):
                G = cap.get((k, ch, ci), 0)
                if G == 0:
                    continue
                scol_of[(ch, ci)] = col
                col += G // 16
                for j in range(G // 128):
                    slot_of[(ch, ci, j)] = slot
                    slot += 1
            if slot > lo:
                scatters.append((lo, slot, soff))
        plan_scol = col
        # gather layout: ch-major, class-minor (separate column region)
        for ch in range(nchk):
            chunks = []
            gci = []
            for ci in range(maxcls):
                G = cap.get((k, ch, ci), 0)
                if G == 0:
                    continue
                gci.append((ci, col, G))
                col += G // 16
                for j in range(G // 128):
                    chunks.append(slot_of[(ch, ci, j)])
            if chunks:
                goff = gci[0][1]
                gathers.append((ch, len(chunks), goff, chunks))
                for (ci, gc, G) in gci:
                    segs[(k, ch, ci)] = (gc, scol_of[(ch, ci)], G)
        plan.append({"gathers": gathers, "scatters": scatters, "nslots": slot})
    ncols16 = col

    # ---- per-core index planes ----
    gidx_all = []
    sidx_all = []
    for c in range(NCORES):
        gplane = np.zeros((128, ncols16), np.int16)
        splane = np.zeros((128, ncols16), np.int16)
        for (k, ch, ci), (gcol, scol, G) in segs.items():
            sel = sel_all.get((c, k, ch, ci), np.zeros(0, np.int64))
            n = len(sel)
            gi = np.zeros(G, np.int16)
            si = np.empty(G, np.int16)
            gi[:n] = (in_idx[k][sel] - ch * CHK).astype(np.int16)
            si[:n] = (out_idx[k][sel] - c * rows_per_core).astype(np.int16)
            si[n:] = rows_per_core + (np.arange(G - n) % TRASH)
            gplane[:, gcol:gcol + G // 16] = _wrap_idx_cols(gi)
            splane[:, scol:scol + G // 16] = _wrap_idx_cols(si)
        gidx_all.append(gplane)
        sidx_all.append(splane)

    nc = _build_program(K, C, nchk, plan, rows_out, ncols16)
    global LAST_NC, LAST_IN_MAPS
    LAST_NC = nc

    in_maps = [{
        "feats_hl": feats_hl,
        "gidx": gidx_all[c],
        "sidx": sidx_all[c],
        "whh": whh,
        "wll": wll,
        "ident": ident,
    } for c in range(NCORES)]
    LAST_IN_MAPS = in_maps

    if BUILD_ONLY:
        return np.zeros((N, C), np.float32)

    if SIM:
        from concourse import bass_interp
        sim = bass_interp.MultiCoreSim(nc, NCORES)
        for c in range(NCORES):
            for name, arr in in_maps[c].items():
                sim.cores[c].tensor(name)[:] = arr
            sim.cores[c].tensor("out")[:] = 0
        sim.simulate()
        outs = [np.array(sim.cores[c].tensor("out")) for c in range(NCORES)]
    else:
        res = run_bass_kernel_spmd(nc, in_maps, list(range(NCORES)))
        global LAST_RESULTS
        LAST_RESULTS = res
        outs = [res.results[c]["out"] for c in range(NCORES)]

    out = np.concatenate([o[:rows_per_core] for o in outs], axis=0)[:N]
    return np.ascontiguousarray(out, dtype=np.float32)



# revision 2
# speedup vs baseline: 4.3297x; 4.3297x over previous
"""Sparse 3D conv (gather -> per-offset matmul -> scatter-add) on 8 Trainium2
NeuronCores — transfer-optimized v2.

The axon tunnel moves ~50 MB/s, so the warm-exec wall is dominated by
host<->device bytes, not device compute. v2 minimizes transferred bytes:

  - feats are sharded (12500 rows/core) as fp16 and AllGathered on-device
    (25.6MB replicated upload -> 1.6MB/core).
  - gather/scatter index planes are shipped non-replicated [16, cols] and
    replicated to 128 partitions on-device; gather and scatter layouts are
    mirrored 1:1 so both planes have identical column structure.
  - classes (duplicate-bin ranks) are computed within (core,k,ch) and scatter
    instructions never span a (k,ch,ci) segment, cutting padding vs the
    within-(core,k) scheme.
  - weights (fp16, hi only), the PE-transpose identity (fp16 bits), and the
    feats shard are packed INTO the same int16 plane tensor (fewer transfers).
  - output is fp16 (internal f32 scatter accumulator, converted at the end).
  - scatter padding uses negative indices (ignored tail) -> no trash rows.

Compute per entry: gather 256B fp16 row -> PE transpose (sliced to the 64
real feature cols; junk cols never enter the PE) -> single fp16 matmul with
W[k] accumulated in f32 PSUM -> staged -> CCE DMA scatter-add (f32).
"""
import sys

if "/opt/trn_rl_repo" not in sys.path:
    sys.path.insert(0, "/opt/trn_rl_repo")

import numpy as np

from concourse import tile, bacc
from concourse import mybir
from concourse.bass_utils import run_bass_kernel_spmd
from concourse.library_config import mlp

F16 = np.float16
NCORES = 8
CHK = 25000          # gather source chunk rows (int16 index limit)
SIM = False          # run in CoreSim (MultiCoreSim) instead of hardware
BUILD_ONLY = False   # build+compile only; stash nc/in_maps and return zeros

LAST_RESULTS = None
LAST_NC = None
LAST_IN_MAPS = None

# plane fixed regions (in int16 columns), after the two index regions
IDENT_COLS = 1024          # [128,128] fp16 identity bits
TRASH = 1024               # trash rows: padding entry at in-instruction
                           # position p scatters to rows_pc + p (unique)


def _wrap16(idx16):
    """[G] int16 -> [16, G//16] wrapped (j -> [j%16, j//16])."""
    n = idx16.shape[0]
    return np.asarray(idx16, np.int16).reshape(n // 16, 16).T


def _build_program(K, C, N, nchk, plan, max_slots, ncols_i, w_cols, f_cols):
    rows_pc = N // NCORES
    pcols = 2 * ncols_i + IDENT_COLS + w_cols + f_cols
    ioff = 2 * ncols_i
    woff = ioff + IDENT_COLS
    foff = woff + w_cols

    nc = bacc.Bacc(None, target_bir_lowering=False, debug=False,
                   num_swdge_queues=1)

    plane_d = nc.dram_tensor("plane", [16, pcols], mybir.dt.int16,
                             kind="ExternalInput")
    out_d = nc.dram_tensor("out", [rows_pc, C], mybir.dt.float16,
                           kind="ExternalOutput")

    with tile.TileContext(nc) as tc:
        with (
            tc.tile_pool(name="dram", bufs=1, space="DRAM") as dpool,
            tc.tile_pool(name="const", bufs=1) as cpool,
            tc.tile_pool(name="xt", bufs=3) as xt_pool,
            tc.tile_pool(name="xtc", bufs=8) as xtc_pool,
            tc.tile_pool(name="yk", bufs=2) as y_pool,
            tc.tile_pool(name="cvt", bufs=3) as cvt_pool,
            tc.tile_pool(name="pst", bufs=4, space="PSUM") as pst_pool,
            tc.tile_pool(name="psy", bufs=4, space="PSUM") as psy_pool,
        ):
            nc.gpsimd.load_library(mlp)

            bounce = dpool.tile([16, rows_pc * C // 16], mybir.dt.float16)
            gathered = dpool.tile([N, C], mybir.dt.float16)
            feats_pad = dpool.tile([N, 2 * C], mybir.dt.float16)
            out_acc = dpool.tile([rows_pc + TRASH, C], mybir.dt.float32)

            # ---- constants from the plane ----
            gidx_t = cpool.tile([128, ncols_i], mybir.dt.int16)
            sidx_t = cpool.tile([128, ncols_i], mybir.dt.int16)
            for r in range(8):
                nc.sync.dma_start(gidx_t[16 * r:16 * (r + 1), :],
                                  plane_d[:, 0:ncols_i])
                nc.sync.dma_start(sidx_t[16 * r:16 * (r + 1), :],
                                  plane_d[:, ncols_i:2 * ncols_i])
            ident_t = cpool.tile([128, 128], mybir.dt.int16)
            nc.sync.dma_start(ident_t[:], plane_d[:, ioff:ioff + IDENT_COLS])
            ident_f = ident_t[:].bitcast(mybir.dt.float16)
            w_t = cpool.tile([C, K * C], mybir.dt.int16)
            nc.sync.dma_start(w_t[:], plane_d[:, woff:woff + w_cols])
            w_f = w_t[:].bitcast(mybir.dt.float16)

            # ---- feats: shard -> allgather -> expand to 256B rows ----
            nc.sync.dma_start(
                bounce[:],
                plane_d[:, foff:foff + f_cols].bitcast(mybir.dt.float16))
            nc.gpsimd.collective_compute(
                "AllGather",
                mybir.AluOpType.bypass,
                replica_groups=[list(range(NCORES))],
                ins=[bounce[:].opt()],
                outs=[gathered[:].opt()],
            )

            def fill_rows(dst_fn, total, src3, nsl):
                """dst_fn(i, h) -> AP over rows [i,i+h) with last dim C."""
                step = 128 * nsl
                i = 0
                while i < total:
                    h = min(step, total - i)
                    full = h // 128 * 128
                    if full:
                        nc.sync.dma_start(dst_fn(i, full),
                                          src3[:, 0:full // 128, :])
                    if h - full:
                        nc.sync.dma_start(dst_fn(i + full, h - full),
                                          src3[0:h - full, 0, :])
                    i += h

            zero16 = cpool.tile([128, 8, C], mybir.dt.float16)
            nc.vector.memset(zero16[:], 0.0)
            fill_rows(lambda i, h: feats_pad[i:i + h, C:2 * C], N, zero16, 8)
            for ch in range(nchk):
                lo, hi = ch * CHK, min((ch + 1) * CHK, N)
                nc.sync.dma_start(feats_pad[lo:hi, 0:C], gathered[lo:hi, :])

            # ---- zero the f32 accumulator ----
            zero_t = cpool.tile([128, 8, C], mybir.dt.float32)
            nc.vector.memset(zero_t[:], 0.0)
            fill_rows(lambda i, h: out_acc[i:i + h, :], rows_pc + TRASH,
                      zero_t, 8)

            # ---- main loop ----
            ncd = 0
            for k in range(K):
                p = plan[k]
                if p["nslots"] == 0:
                    continue
                y_k = y_pool.tile([128, max_slots, C], mybir.dt.float32,
                                  tag="yk")
                for (ch, m_all, goff) in p["gathers"]:
                    for sub in range(0, m_all, 8):
                        m = min(8, m_all - sub)
                        off = goff + sub * 8
                        slot0 = p["slot0"][(ch, m_all, goff)] + sub
                        G = m * 128
                        xt = xt_pool.tile([128, m, 2 * C], mybir.dt.float16,
                                          tag="xt")
                        nc.gpsimd.dma_gather(
                            xt[:],
                            feats_pad[ch * CHK:min((ch + 1) * CHK, N), :],
                            gidx_t[:, off:off + G // 16],
                            G, G, 2 * C, queue_num=0,
                        )
                        for j in range(m):
                            t_ps = pst_pool.tile([C, 128], mybir.dt.float16,
                                                 tag="pt")
                            nc.tensor.transpose(t_ps[:], xt[:, j, 0:C],
                                                ident_f)
                            xt_col = xtc_pool.tile([C, 128], mybir.dt.float16,
                                                   tag="xtc")
                            if ncd % 2 == 0:
                                nc.vector.tensor_copy(xt_col[:], t_ps[:])
                            else:
                                nc.scalar.copy(xt_col[:], t_ps[:])
                            y_ps = psy_pool.tile([128, C], mybir.dt.float32,
                                                 tag="py")
                            nc.tensor.matmul(y_ps[:], xt_col[:],
                                             w_f[:, k * C:(k + 1) * C],
                                             start=True, stop=True)
                            s = slot0 + j
                            if ncd % 2 == 0:
                                nc.scalar.copy(y_k[:, s, :], y_ps[:])
                            else:
                                nc.vector.tensor_copy(y_k[:, s, :], y_ps[:])
                            ncd += 1
                for (lo_all, hi_all, soff_all) in p["scatters"]:
                    for lo in range(lo_all, hi_all, 8):
                        hi = min(lo + 8, hi_all)
                        soff = soff_all + (lo - lo_all) * 8
                        G = (hi - lo) * 128
                        nc.gpsimd.dma_scatter_add(
                            out_acc[:], y_k[:, lo:hi, :],
                            sidx_t[:, soff:soff + G // 16],
                            G, G, C, queue_num=0,
                        )

            # ---- f32 accumulator -> fp16 output ----
            crows = 128 * 8
            for i in range(0, rows_pc, crows):
                h = min(crows, rows_pc - i)
                full = h // 128 * 128
                rem = h - full
                tf = cvt_pool.tile([128, 8, C], mybir.dt.float32, tag="cf")
                th = cvt_pool.tile([128, 8, C], mybir.dt.float16, tag="ch")
                if full:
                    nc.sync.dma_start(tf[:, 0:full // 128, :],
                                      out_acc[i:i + full, :])
                    nc.vector.tensor_copy(th[:, 0:full // 128, :],
                                          tf[:, 0:full // 128, :])
                    nc.sync.dma_start(out_d[i:i + full, :],
                                      th[:, 0:full // 128, :])
                if rem:
                    nc.sync.dma_start(tf[0:rem, 7, :],
                                      out_acc[i + full:i + h, :])
                    nc.scalar.copy(th[0:rem, 7, :], tf[0:rem, 7, :])
                    nc.sync.dma_start(out_d[i + full:i + h, :],
                                      th[0:rem, 7, :])

    nc.compile()
    return nc


def _route(in_idx, out_idx, rows_pc, K, nchk):
    """Entry lists per (core, k, ch, ci); ci = duplicate-bin rank within
    (core, k, ch)."""
    sel_all = {}
    core_of = out_idx // rows_pc          # [K, M]
    ch_of = in_idx // CHK
    for k in range(K):
        for c in range(NCORES):
            m = np.nonzero(core_of[k] == c)[0]
            if len(m) == 0:
                continue
            chv = ch_of[k][m]
            bins = out_idx[k][m] - c * rows_pc
            for ch in range(nchk):
                mm = m[chv == ch]
                if len(mm) == 0:
                    continue
                b = out_idx[k][mm] - c * rows_pc
                order = np.argsort(b, kind="stable")
                sb = b[order]
                grp = np.zeros(len(sb), np.int64)
                if len(sb) > 1:
                    new = np.nonzero(sb[1:] != sb[:-1])[0] + 1
                    starts = np.zeros(len(sb), np.int64)
                    starts[new] = new
                    grp = np.maximum.accumulate(starts)
                rank_sorted = np.arange(len(sb)) - grp
                rank = np.empty(len(mm), np.int64)
                rank[order] = rank_sorted
                for ci in range(int(rank.max()) + 1):
                    sel = mm[rank == ci]
                    if len(sel):
                        sel_all[(c, k, ch, ci)] = sel
    return sel_all


def _prepare(feats, Wk, in_idx, out_idx):
    """Host-side routing + shared plan + per-core plane assembly."""
    N, C = feats.shape
    K, M = in_idx.shape
    rows_pc = N // NCORES
    nchk = (N + CHK - 1) // CHK

    sel_all = _route(in_idx, out_idx, rows_pc, K, nchk)

    # static caps per (k, ch, ci) = roundup128(max over cores)
    cap = {}
    maxci = {}
    for (c, k, ch, ci), sel in sel_all.items():
        key = (k, ch, ci)
        cap[key] = max(cap.get(key, 0), len(sel))
        maxci[(k, ch)] = max(maxci.get((k, ch), -1), ci)
    cap = {key: (v + 127) // 128 * 128 for key, v in cap.items()}

    # ---- shared plan: mirrored gather/scatter layout ----
    # per k: slots ordered (ch, ci, j); gather groups per ch (span ci),
    # scatter batches per (ch, ci) segment.
    plan = []
    col = 0                    # column cursor (16-entry units), shared layout
    segs = {}                  # (k, ch, ci) -> col
    max_slots = 0
    for k in range(K):
        gathers = []
        scatters = []
        slot0 = {}
        slot = 0
        for ch in range(nchk):
            if (k, ch) not in maxci:
                continue
            gcol = col
            nch_slots = 0
            for ci in range(maxci[(k, ch)] + 1):
                G = cap.get((k, ch, ci), 0)
                if G == 0:
                    continue
                segs[(k, ch, ci)] = col
                scatters.append((slot + nch_slots,
                                 slot + nch_slots + G // 128, col * 16 // 16))
                col += G // 16
                nch_slots += G // 128
            g = (ch, nch_slots, gcol)
            gathers.append(g)
            slot0[g] = slot
            slot += nch_slots
        max_slots = max(max_slots, slot)
        plan.append({"gathers": gathers, "scatters": scatters,
                     "slot0": slot0, "nslots": slot})
    ncols_i = col

    # ---- fixed plane regions ----
    w16 = np.concatenate([Wk[k].astype(F16) for k in range(K)], axis=1)
    w_plane = w16.view(np.int16).reshape(16, -1)       # [C,K*C] -> [16, cols]
    w_cols = w_plane.shape[1]
    ident_plane = np.eye(128, dtype=F16).view(np.int16).reshape(16, 1024)
    f16s = feats.astype(F16)
    f_cols = rows_pc * C // 16

    # ---- per-core planes ----
    in_maps = []
    for c in range(NCORES):
        gplane = np.zeros((16, ncols_i), np.int16)
        splane = np.zeros((16, ncols_i), np.int16)
        for (k, ch, ci), col0 in segs.items():
            G = cap[(k, ch, ci)]
            sel = sel_all.get((c, k, ch, ci), np.zeros(0, np.int64))
            n = len(sel)
            gi = np.zeros(G, np.int16)
            pad_pos = np.arange(n, G)
            si = np.empty(G, np.int16)
            si[n:] = (rows_pc + pad_pos % 1024).astype(np.int16)
            gi[:n] = (in_idx[k][sel] - ch * CHK).astype(np.int16)
            si[:n] = (out_idx[k][sel] - c * rows_pc).astype(np.int16)
            gplane[:, col0:col0 + G // 16] = _wrap16(gi)
            splane[:, col0:col0 + G // 16] = _wrap16(si)
        fsh = f16s[c * rows_pc:(c + 1) * rows_pc].reshape(16, f_cols)
        fsh = fsh.view(np.int16)
        plane = np.concatenate(
            [gplane, splane, ident_plane, w_plane, fsh], axis=1)
        in_maps.append({"plane": np.ascontiguousarray(plane)})

    return {"plan": plan, "in_maps": in_maps, "max_slots": max_slots,
            "ncols_i": ncols_i, "w_cols": w_cols, "f_cols": f_cols,
            "N": N, "C": C, "K": K, "nchk": nchk, "rows_pc": rows_pc,
            "segs": segs, "cap": cap}


def kernel(feats, kernel, in_idx, out_idx):
    feats = np.asarray(feats, np.float32)
    Wk = np.asarray(kernel, np.float32)
    in_idx = np.asarray(in_idx, np.int64)
    out_idx = np.asarray(out_idx, np.int64)

    N, C = feats.shape
    pr = _prepare(feats, Wk, in_idx, out_idx)
    K, nchk, rows_pc = pr["K"], pr["nchk"], pr["rows_pc"]
    in_maps = pr["in_maps"]

    nc = _build_program(K, C, N, nchk, pr["plan"], pr["max_slots"],
                        pr["ncols_i"], pr["w_cols"], pr["f_cols"])
    global LAST_NC, LAST_IN_MAPS
    LAST_NC = nc
    LAST_IN_MAPS = in_maps

    if BUILD_ONLY:
        return np.zeros((N, C), np.float32)

    if SIM:
        from concourse import bass_interp
        sim = bass_interp.MultiCoreSim(nc, NCORES)
        for c in range(NCORES):
            for name, arr in in_maps[c].items():
                sim.cores[c].tensor(name)[:] = arr
            sim.cores[c].tensor("out")[:] = 0
        sim.simulate()
        outs = [np.array(sim.cores[c].tensor("out")) for c in range(NCORES)]
    else:
        res = run_bass_kernel_spmd(nc, in_maps, list(range(NCORES)))
        global LAST_RESULTS
        LAST_RESULTS = res
        outs = [res.results[c]["out"] for c in range(NCORES)]

    out = np.concatenate(outs, axis=0)[:N]
    return np.ascontiguousarray(out, dtype=np.float32)


# revision 4
# speedup vs baseline: 4.7907x; 1.1065x over previous
"""Sparse 3D conv (gather -> per-offset matmul -> scatter-add) on 8 Trainium2
NeuronCores — transfer-optimized v2.

The axon tunnel moves ~50 MB/s, so the warm-exec wall is dominated by
host<->device bytes, not device compute. v2 minimizes transferred bytes:

  - feats are sharded (12500 rows/core) as fp16 and AllGathered on-device
    (25.6MB replicated upload -> 1.6MB/core).
  - gather/scatter index planes are shipped non-replicated [16, cols] and
    replicated to 128 partitions on-device; gather and scatter layouts are
    mirrored 1:1 so both planes have identical column structure.
  - classes (duplicate-bin ranks) are computed within (core,k,ch) and scatter
    instructions never span a (k,ch,ci) segment, cutting padding vs the
    within-(core,k) scheme.
  - weights (fp16, hi only), the PE-transpose identity (fp16 bits), and the
    feats shard are packed INTO the same int16 plane tensor (fewer transfers).
  - output is fp16 (internal f32 scatter accumulator, converted at the end).
  - scatter padding uses negative indices (ignored tail) -> no trash rows.

Compute per entry: gather 256B fp16 row -> PE transpose (sliced to the 64
real feature cols; junk cols never enter the PE) -> single fp16 matmul with
W[k] accumulated in f32 PSUM -> staged -> CCE DMA scatter-add (f32).
"""
import sys

if "/opt/trn_rl_repo" not in sys.path:
    sys.path.insert(0, "/opt/trn_rl_repo")

import numpy as np

from concourse import tile, bacc
from concourse import mybir
from concourse.bass_utils import run_bass_kernel_spmd
from concourse.library_config import mlp

F16 = np.float16
NCORES = 8
CHK = 25000          # gather source chunk rows (int16 index limit)
SIM = False          # run in CoreSim (MultiCoreSim) instead of hardware
BUILD_ONLY = False   # build+compile only; stash nc/in_maps and return zeros

LAST_RESULTS = None
LAST_NC = None
LAST_IN_MAPS = None

# plane fixed regions (in int16 columns), after the two index regions
IDENT_COLS = 1024          # [128,128] fp16 identity bits
TRASH = 1024               # trash rows: padding entry at in-instruction
                           # position p scatters to rows_pc + p (unique)


def _wrap16(idx16):
    """[G] int16 -> [16, G//16] wrapped (j -> [j%16, j//16])."""
    n = idx16.shape[0]
    return np.asarray(idx16, np.int16).reshape(n // 16, 16).T


def _build_program(K, C, N, nchk, plan, max_slots, ncols_i, w_cols, f_cols):
    rows_pc = N // NCORES
    pcols = 2 * ncols_i + IDENT_COLS + w_cols + f_cols
    ioff = 2 * ncols_i
    woff = ioff + IDENT_COLS
    foff = woff + w_cols

    nc = bacc.Bacc(None, target_bir_lowering=False, debug=False,
                   num_swdge_queues=1)

    plane_d = nc.dram_tensor("plane", [16, pcols], mybir.dt.int16,
                             kind="ExternalInput")
    out_d = nc.dram_tensor("out", [rows_pc, C], mybir.dt.float16,
                           kind="ExternalOutput")

    with tile.TileContext(nc) as tc:
        with (
            tc.tile_pool(name="dram", bufs=1, space="DRAM") as dpool,
            tc.tile_pool(name="const", bufs=1) as cpool,
            tc.tile_pool(name="xt", bufs=3) as xt_pool,
            tc.tile_pool(name="xtc", bufs=8) as xtc_pool,
            tc.tile_pool(name="yk", bufs=2) as y_pool,
            tc.tile_pool(name="cvt", bufs=3) as cvt_pool,
            tc.tile_pool(name="pst", bufs=4, space="PSUM") as pst_pool,
            tc.tile_pool(name="psy", bufs=4, space="PSUM") as psy_pool,
        ):
            nc.gpsimd.load_library(mlp)

            bounce = dpool.tile([16, rows_pc * C // 16], mybir.dt.float16)
            gathered = dpool.tile([N, C], mybir.dt.float16)
            feats_pad = dpool.tile([N, 2 * C], mybir.dt.float16)
            out_acc = dpool.tile([rows_pc + TRASH, C], mybir.dt.float32)

            # ---- constants from the plane ----
            gidx_t = cpool.tile([128, ncols_i], mybir.dt.int16)
            sidx_t = cpool.tile([128, ncols_i], mybir.dt.int16)
            for r in range(8):
                nc.sync.dma_start(gidx_t[16 * r:16 * (r + 1), :],
                                  plane_d[:, 0:ncols_i])
                nc.sync.dma_start(sidx_t[16 * r:16 * (r + 1), :],
                                  plane_d[:, ncols_i:2 * ncols_i])
            ident_t = cpool.tile([128, 128], mybir.dt.int16)
            nc.sync.dma_start(ident_t[:], plane_d[:, ioff:ioff + IDENT_COLS])
            ident_f = ident_t[:].bitcast(mybir.dt.float16)
            w_t = cpool.tile([C, K * C], mybir.dt.int16)
            nc.sync.dma_start(w_t[:], plane_d[:, woff:woff + w_cols])
            w_f = w_t[:].bitcast(mybir.dt.float16)

            # ---- feats: shard -> allgather -> expand to 256B rows ----
            nc.sync.dma_start(
                bounce[:],
                plane_d[:, foff:foff + f_cols].bitcast(mybir.dt.float16))
            nc.gpsimd.collective_compute(
                "AllGather",
                mybir.AluOpType.bypass,
                replica_groups=[list(range(NCORES))],
                ins=[bounce[:].opt()],
                outs=[gathered[:].opt()],
            )

            def fill_rows(dst_fn, total, src3, nsl):
                """dst_fn(i, h) -> AP over rows [i,i+h) with last dim C."""
                step = 128 * nsl
                i = 0
                while i < total:
                    h = min(step, total - i)
                    full = h // 128 * 128
                    if full:
                        nc.sync.dma_start(dst_fn(i, full),
                                          src3[:, 0:full // 128, :])
                    if h - full:
                        nc.sync.dma_start(dst_fn(i + full, h - full),
                                          src3[0:h - full, 0, :])
                    i += h

            zero16 = cpool.tile([128, 8, C], mybir.dt.float16)
            nc.vector.memset(zero16[:], 0.0)
            fill_rows(lambda i, h: feats_pad[i:i + h, C:2 * C], N, zero16, 8)
            for ch in range(nchk):
                lo, hi = ch * CHK, min((ch + 1) * CHK, N)
                nc.sync.dma_start(feats_pad[lo:hi, 0:C], gathered[lo:hi, :])

            # ---- zero the f32 accumulator ----
            zero_t = cpool.tile([128, 8, C], mybir.dt.float32)
            nc.vector.memset(zero_t[:], 0.0)
            fill_rows(lambda i, h: out_acc[i:i + h, :], rows_pc + TRASH,
                      zero_t, 8)

            # ---- main loop ----
            ncd = 0
            for k in range(K):
                p = plan[k]
                if p["nslots"] == 0:
                    continue
                y_k = y_pool.tile([128, max_slots, C], mybir.dt.float32,
                                  tag="yk")
                for (ch, m_all, goff) in p["gathers"]:
                    for sub in range(0, m_all, 8):
                        m = min(8, m_all - sub)
                        off = goff + sub * 8
                        slot0 = p["slot0"][(ch, m_all, goff)] + sub
                        G = m * 128
                        xt = xt_pool.tile([128, m, 2 * C], mybir.dt.float16,
                                          tag="xt")
                        nc.gpsimd.dma_gather(
                            xt[:],
                            feats_pad[ch * CHK:min((ch + 1) * CHK, N), :],
                            gidx_t[:, off:off + G // 16],
                            G, G, 2 * C, queue_num=0,
                        )
                        for j in range(m):
                            t_ps = pst_pool.tile([C, 128], mybir.dt.float16,
                                                 tag="pt")
                            nc.tensor.transpose(t_ps[:], xt[:, j, 0:C],
                                                ident_f)
                            xt_col = xtc_pool.tile([C, 128], mybir.dt.float16,
                                                   tag="xtc")
                            if ncd % 2 == 0:
                                nc.vector.tensor_copy(xt_col[:], t_ps[:])
                            else:
                                nc.scalar.copy(xt_col[:], t_ps[:])
                            y_ps = psy_pool.tile([128, C], mybir.dt.float32,
                                                 tag="py")
                            nc.tensor.matmul(y_ps[:], xt_col[:],
                                             w_f[:, k * C:(k + 1) * C],
                                             start=True, stop=True)
                            s = slot0 + j
                            if ncd % 2 == 0:
                                nc.scalar.copy(y_k[:, s, :], y_ps[:])
                            else:
                                nc.vector.tensor_copy(y_k[:, s, :], y_ps[:])
                            ncd += 1
                for (lo_all, hi_all, soff_all) in p["scatters"]:
                    for lo in range(lo_all, hi_all, 8):
                        hi = min(lo + 8, hi_all)
                        soff = soff_all + (lo - lo_all) * 8
                        G = (hi - lo) * 128
                        nc.gpsimd.dma_scatter_add(
                            out_acc[:], y_k[:, lo:hi, :],
                            sidx_t[:, soff:soff + G // 16],
                            G, G, C, queue_num=0,
                        )

            # ---- f32 accumulator -> fp16 output ----
            crows = 128 * 8
            for i in range(0, rows_pc, crows):
                h = min(crows, rows_pc - i)
                full = h // 128 * 128
                rem = h - full
                tf = cvt_pool.tile([128, 8, C], mybir.dt.float32, tag="cf")
                th = cvt_pool.tile([128, 8, C], mybir.dt.float16, tag="ch")
                if full:
                    nc.sync.dma_start(tf[:, 0:full // 128, :],
                                      out_acc[i:i + full, :])
                    nc.vector.tensor_copy(th[:, 0:full // 128, :],
                                          tf[:, 0:full // 128, :])
                    nc.sync.dma_start(out_d[i:i + full, :],
                                      th[:, 0:full // 128, :])
                if rem:
                    nc.sync.dma_start(tf[0:rem, 7, :],
                                      out_acc[i + full:i + h, :])
                    nc.scalar.copy(th[0:rem, 7, :], tf[0:rem, 7, :])
                    nc.sync.dma_start(out_d[i + full:i + h, :],
                                      th[0:rem, 7, :])

    nc.compile()
    return nc


def _route(in_idx, out_idx, rows_pc, K, nchk):
    """Entry lists per (core, k, ch, ci); ci = duplicate-bin rank within
    (core, k, ch)."""
    sel_all = {}
    core_of = out_idx // rows_pc          # [K, M]
    ch_of = in_idx // CHK
    for k in range(K):
        for c in range(NCORES):
            m = np.nonzero(core_of[k] == c)[0]
            if len(m) == 0:
                continue
            chv = ch_of[k][m]
            bins = out_idx[k][m] - c * rows_pc
            for ch in range(nchk):
                mm = m[chv == ch]
                if len(mm) == 0:
                    continue
                b = out_idx[k][mm] - c * rows_pc
                order = np.argsort(b, kind="stable")
                sb = b[order]
                grp = np.zeros(len(sb), np.int64)
                if len(sb) > 1:
                    new = np.nonzero(sb[1:] != sb[:-1])[0] + 1
                    starts = np.zeros(len(sb), np.int64)
                    starts[new] = new
                    grp = np.maximum.accumulate(starts)
                rank_sorted = np.arange(len(sb)) - grp
                rank = np.empty(len(mm), np.int64)
                rank[order] = rank_sorted
                for ci in range(int(rank.max()) + 1):
                    sel = mm[rank == ci]
                    if len(sel):
                        sel_all[(c, k, ch, ci)] = sel
    return sel_all


def _prepare(feats, Wk, in_idx, out_idx):
    """Host-side routing + shared plan + per-core plane assembly."""
    N, C = feats.shape
    K, M = in_idx.shape
    rows_pc = N // NCORES
    nchk = (N + CHK - 1) // CHK

    sel_all = _route(in_idx, out_idx, rows_pc, K, nchk)

    # static caps per (k, ch, ci) = roundup128(max over cores)
    cap = {}
    maxci = {}
    for (c, k, ch, ci), sel in sel_all.items():
        key = (k, ch, ci)
        cap[key] = max(cap.get(key, 0), len(sel))
        maxci[(k, ch)] = max(maxci.get((k, ch), -1), ci)
    cap = {key: (v + 127) // 128 * 128 for key, v in cap.items()}

    # ---- shared plan: mirrored gather/scatter layout ----
    # per k: slots ordered (ch, ci, j); gather groups per ch (span ci),
    # scatter batches per (ch, ci) segment.
    plan = []
    col = 0                    # column cursor (16-entry units), shared layout
    segs = {}                  # (k, ch, ci) -> col
    max_slots = 0
    for k in range(K):
        gathers = []
        scatters = []
        slot0 = {}
        slot = 0
        for ch in range(nchk):
            if (k, ch) not in maxci:
                continue
            gcol = col
            nch_slots = 0
            for ci in range(maxci[(k, ch)] + 1):
                G = cap.get((k, ch, ci), 0)
                if G == 0:
                    continue
                segs[(k, ch, ci)] = col
                scatters.append((slot + nch_slots,
                                 slot + nch_slots + G // 128, col * 16 // 16))
                col += G // 16
                nch_slots += G // 128
            g = (ch, nch_slots, gcol)
            gathers.append(g)
            slot0[g] = slot
            slot += nch_slots
        max_slots = max(max_slots, slot)
        plan.append({"gathers": gathers, "scatters": scatters,
                     "slot0": slot0, "nslots": slot})
    ncols_i = col

    # ---- fixed plane regions ----
    w16 = np.concatenate([Wk[k].astype(F16) for k in range(K)], axis=1)
    w_plane = w16.view(np.int16).reshape(16, -1)       # [C,K*C] -> [16, cols]
    w_cols = w_plane.shape[1]
    ident_plane = np.eye(128, dtype=F16).view(np.int16).reshape(16, 1024)
    f16s = feats.astype(F16)
    f_cols = rows_pc * C // 16

    # ---- per-core planes ----
    in_maps = []
    for c in range(NCORES):
        gplane = np.zeros((16, ncols_i), np.int16)
        splane = np.zeros((16, ncols_i), np.int16)
        for (k, ch, ci), col0 in segs.items():
            G = cap[(k, ch, ci)]
            sel = sel_all.get((c, k, ch, ci), np.zeros(0, np.int64))
            n = len(sel)
            gi = np.zeros(G, np.int16)
            pad_pos = np.arange(n, G)
            si = np.empty(G, np.int16)
            si[n:] = (rows_pc + pad_pos % 1024).astype(np.int16)
            gi[:n] = (in_idx[k][sel] - ch * CHK).astype(np.int16)
            si[:n] = (out_idx[k][sel] - c * rows_pc).astype(np.int16)
            gplane[:, col0:col0 + G // 16] = _wrap16(gi)
            splane[:, col0:col0 + G // 16] = _wrap16(si)
        fsh = f16s[c * rows_pc:(c + 1) * rows_pc].reshape(16, f_cols)
        fsh = fsh.view(np.int16)
        plane = np.concatenate(
            [gplane, splane, ident_plane, w_plane, fsh], axis=1)
        in_maps.append({"plane": np.ascontiguousarray(plane)})

    return {"plan": plan, "in_maps": in_maps, "max_slots": max_slots,
            "ncols_i": ncols_i, "w_cols": w_cols, "f_cols": f_cols,
            "N": N, "C": C, "K": K, "nchk": nchk, "rows_pc": rows_pc,
            "segs": segs, "cap": cap}


def kernel(feats, kernel, in_idx, out_idx):
    feats = np.asarray(feats, np.float32)
    Wk = np.asarray(kernel, np.float32)
    in_idx = np.asarray(in_idx, np.int64)
    out_idx = np.asarray(out_idx, np.int64)

    N, C = feats.shape
    pr = _prepare(feats, Wk, in_idx, out_idx)
    K, nchk, rows_pc = pr["K"], pr["nchk"], pr["rows_pc"]
    in_maps = pr["in_maps"]

    nc = _build_program(K, C, N, nchk, pr["plan"], pr["max_slots"],
                        pr["ncols_i"], pr["w_cols"], pr["f_cols"])
    global LAST_NC, LAST_IN_MAPS
    LAST_NC = nc
    LAST_IN_MAPS = in_maps

    if BUILD_ONLY:
        return np.zeros((N, C), np.float32)

    if SIM:
        from concourse import bass_interp
        sim = bass_interp.MultiCoreSim(nc, NCORES)
        for c in range(NCORES):
            for name, arr in in_maps[c].items():
                sim.cores[c].tensor(name)[:] = arr
            sim.cores[c].tensor("out")[:] = 0
        sim.simulate()
        outs = [np.array(sim.cores[c].tensor("out")) for c in range(NCORES)]
    else:
        res = run_bass_kernel_spmd(nc, in_maps, list(range(NCORES)))
        global LAST_RESULTS
        LAST_RESULTS = res
        outs = [res.results[c]["out"] for c in range(NCORES)]

    out = np.concatenate(outs, axis=0)[:N]
    return np.ascontiguousarray(out, dtype=np.float32)


# revision 5
# speedup vs baseline: 4.8974x; 1.0223x over previous
"""Sparse 3D conv (gather -> per-offset matmul -> scatter-add) on 8 Trainium2
NeuronCores — transfer-optimized v2.

The axon tunnel moves ~50 MB/s, so the warm-exec wall is dominated by
host<->device bytes, not device compute. v2 minimizes transferred bytes:

  - feats are sharded (12500 rows/core) as fp16 and AllGathered on-device
    (25.6MB replicated upload -> 1.6MB/core).
  - gather/scatter index planes are shipped non-replicated [16, cols] and
    replicated to 128 partitions on-device; gather and scatter layouts are
    mirrored 1:1 so both planes have identical column structure.
  - classes (duplicate-bin ranks) are computed within (core,k,ch) and scatter
    instructions never span a (k,ch,ci) segment, cutting padding vs the
    within-(core,k) scheme.
  - weights (fp16, hi only), the PE-transpose identity (fp16 bits), and the
    feats shard are packed INTO the same int16 plane tensor (fewer transfers).
  - output is fp16 (internal f32 scatter accumulator, converted at the end).
  - scatter padding uses negative indices (ignored tail) -> no trash rows.

Compute per entry: gather 256B fp16 row -> PE transpose (sliced to the 64
real feature cols; junk cols never enter the PE) -> single fp16 matmul with
W[k] accumulated in f32 PSUM -> staged -> CCE DMA scatter-add (f32).
"""
import sys

if "/opt/trn_rl_repo" not in sys.path:
    sys.path.insert(0, "/opt/trn_rl_repo")

import numpy as np

from concourse import tile, bacc
from concourse import mybir
from concourse.bass_utils import run_bass_kernel_spmd
from concourse.library_config import mlp

F16 = np.float16
NCORES = 8
CHK = 25000          # gather source chunk rows (int16 index limit)
SIM = False          # run in CoreSim (MultiCoreSim) instead of hardware
BUILD_ONLY = False   # build+compile only; stash nc/in_maps and return zeros

LAST_RESULTS = None
LAST_NC = None
LAST_IN_MAPS = None

# plane fixed regions (in int16 columns), after the two index regions
IDENT_COLS = 1024          # [128,128] fp16 identity bits
TRASH = 1024               # trash rows: padding entry at in-instruction
                           # position p scatters to rows_pc + p (unique)


def _wrap16(idx16):
    """[G] int16 -> [16, G//16] wrapped (j -> [j%16, j//16])."""
    n = idx16.shape[0]
    return np.asarray(idx16, np.int16).reshape(n // 16, 16).T


def _build_program(K, C, N, nchk, plan, max_slots, ncols_i, w_cols, f_cols):
    rows_pc = N // NCORES
    pcols = 2 * ncols_i + IDENT_COLS + w_cols + f_cols
    ioff = 2 * ncols_i
    woff = ioff + IDENT_COLS
    foff = woff + w_cols

    nc = bacc.Bacc(None, target_bir_lowering=False, debug=False,
                   num_swdge_queues=2)

    plane_d = nc.dram_tensor("plane", [16, pcols], mybir.dt.int16,
                             kind="ExternalInput")
    out_d = nc.dram_tensor("out", [rows_pc, C], mybir.dt.float16,
                           kind="ExternalOutput")

    with tile.TileContext(nc) as tc:
        with (
            tc.tile_pool(name="dram", bufs=1, space="DRAM") as dpool,
            tc.tile_pool(name="const", bufs=1) as cpool,
            tc.tile_pool(name="xt", bufs=3) as xt_pool,
            tc.tile_pool(name="xtc", bufs=8) as xtc_pool,
            tc.tile_pool(name="yk", bufs=2) as y_pool,
            tc.tile_pool(name="cvt", bufs=3) as cvt_pool,
            tc.tile_pool(name="pst", bufs=4, space="PSUM") as pst_pool,
            tc.tile_pool(name="psy", bufs=4, space="PSUM") as psy_pool,
        ):
            nc.gpsimd.load_library(mlp)

            bounce = dpool.tile([16, rows_pc * C // 16], mybir.dt.float16)
            gathered = dpool.tile([N, C], mybir.dt.float16)
            feats_pad = dpool.tile([N, 2 * C], mybir.dt.float16)
            out_acc = dpool.tile([rows_pc + TRASH, C], mybir.dt.float32)

            # ---- constants from the plane ----
            gidx_t = cpool.tile([128, ncols_i], mybir.dt.int16)
            sidx_t = cpool.tile([128, ncols_i], mybir.dt.int16)
            for r in range(8):
                nc.sync.dma_start(gidx_t[16 * r:16 * (r + 1), :],
                                  plane_d[:, 0:ncols_i])
                nc.sync.dma_start(sidx_t[16 * r:16 * (r + 1), :],
                                  plane_d[:, ncols_i:2 * ncols_i])
            ident_t = cpool.tile([128, 128], mybir.dt.int16)
            nc.sync.dma_start(ident_t[:], plane_d[:, ioff:ioff + IDENT_COLS])
            ident_f = ident_t[:].bitcast(mybir.dt.float16)
            w_t = cpool.tile([C, K * C], mybir.dt.int16)
            nc.sync.dma_start(w_t[:], plane_d[:, woff:woff + w_cols])
            w_f = w_t[:].bitcast(mybir.dt.float16)

            # ---- feats: shard -> allgather -> expand to 256B rows ----
            nc.sync.dma_start(
                bounce[:],
                plane_d[:, foff:foff + f_cols].bitcast(mybir.dt.float16))
            nc.gpsimd.collective_compute(
                "AllGather",
                mybir.AluOpType.bypass,
                replica_groups=[list(range(NCORES))],
                ins=[bounce[:].opt()],
                outs=[gathered[:].opt()],
            )

            def fill_rows(dst_fn, total, src3, nsl):
                """dst_fn(i, h) -> AP over rows [i,i+h) with last dim C."""
                step = 128 * nsl
                i = 0
                while i < total:
                    h = min(step, total - i)
                    full = h // 128 * 128
                    if full:
                        nc.sync.dma_start(dst_fn(i, full),
                                          src3[:, 0:full // 128, :])
                    if h - full:
                        nc.sync.dma_start(dst_fn(i + full, h - full),
                                          src3[0:h - full, 0, :])
                    i += h

            zero16 = cpool.tile([128, 8, C], mybir.dt.float16)
            nc.vector.memset(zero16[:], 0.0)
            fill_rows(lambda i, h: feats_pad[i:i + h, C:2 * C], N, zero16, 8)
            for ch in range(nchk):
                lo, hi = ch * CHK, min((ch + 1) * CHK, N)
                nc.sync.dma_start(feats_pad[lo:hi, 0:C], gathered[lo:hi, :])

            # ---- zero the f32 accumulator ----
            zero_t = cpool.tile([128, 8, C], mybir.dt.float32)
            nc.vector.memset(zero_t[:], 0.0)
            fill_rows(lambda i, h: out_acc[i:i + h, :], rows_pc + TRASH,
                      zero_t, 8)

            # ---- main loop ----
            ncd = 0
            for k in range(K):
                p = plan[k]
                if p["nslots"] == 0:
                    continue
                y_k = y_pool.tile([128, max_slots, C], mybir.dt.float32,
                                  tag="yk")
                for (ch, m_all, goff) in p["gathers"]:
                    for sub in range(0, m_all, 8):
                        m = min(8, m_all - sub)
                        off = goff + sub * 8
                        slot0 = p["slot0"][(ch, m_all, goff)] + sub
                        G = m * 128
                        xt = xt_pool.tile([128, m, 2 * C], mybir.dt.float16,
                                          tag="xt")
                        nc.gpsimd.dma_gather(
                            xt[:],
                            feats_pad[ch * CHK:min((ch + 1) * CHK, N), :],
                            gidx_t[:, off:off + G // 16],
                            G, G, 2 * C, queue_num=0,
                        )
                        for j in range(m):
                            t_ps = pst_pool.tile([C, 128], mybir.dt.float16,
                                                 tag="pt")
                            nc.tensor.transpose(t_ps[:], xt[:, j, 0:C],
                                                ident_f)
                            xt_col = xtc_pool.tile([C, 128], mybir.dt.float16,
                                                   tag="xtc")
                            if ncd % 2 == 0:
                                nc.vector.tensor_copy(xt_col[:], t_ps[:])
                            else:
                                nc.scalar.copy(xt_col[:], t_ps[:])
                            y_ps = psy_pool.tile([128, C], mybir.dt.float32,
                                                 tag="py")
                            nc.tensor.matmul(y_ps[:], xt_col[:],
                                             w_f[:, k * C:(k + 1) * C],
                                             start=True, stop=True)
                            s = slot0 + j
                            if ncd % 2 == 0:
                                nc.scalar.copy(y_k[:, s, :], y_ps[:])
                            else:
                                nc.vector.tensor_copy(y_k[:, s, :], y_ps[:])
                            ncd += 1
                for (lo_all, hi_all, soff_all) in p["scatters"]:
                    for lo in range(lo_all, hi_all, 8):
                        hi = min(lo + 8, hi_all)
                        soff = soff_all + (lo - lo_all) * 8
                        G = (hi - lo) * 128
                        nc.gpsimd.dma_scatter_add(
                            out_acc[:], y_k[:, lo:hi, :],
                            sidx_t[:, soff:soff + G // 16],
                            G, G, C, queue_num=1,
                        )

            # ---- f32 accumulator -> fp16 output ----
            crows = 128 * 8
            for i in range(0, rows_pc, crows):
                h = min(crows, rows_pc - i)
                full = h // 128 * 128
                rem = h - full
                tf = cvt_pool.tile([128, 8, C], mybir.dt.float32, tag="cf")
                th = cvt_pool.tile([128, 8, C], mybir.dt.float16, tag="ch")
                if full:
                    nc.sync.dma_start(tf[:, 0:full // 128, :],
                                      out_acc[i:i + full, :])
                    nc.vector.tensor_copy(th[:, 0:full // 128, :],
                                          tf[:, 0:full // 128, :])
                    nc.sync.dma_start(out_d[i:i + full, :],
                                      th[:, 0:full // 128, :])
                if rem:
                    nc.sync.dma_start(tf[0:rem, 7, :],
                                      out_acc[i + full:i + h, :])
                    nc.scalar.copy(th[0:rem, 7, :], tf[0:rem, 7, :])
                    nc.sync.dma_start(out_d[i + full:i + h, :],
                                      th[0:rem, 7, :])

    nc.compile()
    return nc


def _route(in_idx, out_idx, rows_pc, K, nchk):
    """Entry lists per (core, k, ch, ci); ci = duplicate-bin rank within
    (core, k, ch)."""
    sel_all = {}
    core_of = out_idx // rows_pc          # [K, M]
    ch_of = in_idx // CHK
    for k in range(K):
        for c in range(NCORES):
            m = np.nonzero(core_of[k] == c)[0]
            if len(m) == 0:
                continue
            chv = ch_of[k][m]
            bins = out_idx[k][m] - c * rows_pc
            for ch in range(nchk):
                mm = m[chv == ch]
                if len(mm) == 0:
                    continue
                b = out_idx[k][mm] - c * rows_pc
                order = np.argsort(b, kind="stable")
                sb = b[order]
                grp = np.zeros(len(sb), np.int64)
                if len(sb) > 1:
                    new = np.nonzero(sb[1:] != sb[:-1])[0] + 1
                    starts = np.zeros(len(sb), np.int64)
                    starts[new] = new
                    grp = np.maximum.accumulate(starts)
                rank_sorted = np.arange(len(sb)) - grp
                rank = np.empty(len(mm), np.int64)
                rank[order] = rank_sorted
                for ci in range(int(rank.max()) + 1):
                    sel = mm[rank == ci]
                    if len(sel):
                        sel_all[(c, k, ch, ci)] = sel
    return sel_all


def _prepare(feats, Wk, in_idx, out_idx):
    """Host-side routing + shared plan + per-core plane assembly."""
    N, C = feats.shape
    K, M = in_idx.shape
    rows_pc = N // NCORES
    nchk = (N + CHK - 1) // CHK

    sel_all = _route(in_idx, out_idx, rows_pc, K, nchk)

    # static caps per (k, ch, ci) = roundup128(max over cores)
    cap = {}
    maxci = {}
    for (c, k, ch, ci), sel in sel_all.items():
        key = (k, ch, ci)
        cap[key] = max(cap.get(key, 0), len(sel))
        maxci[(k, ch)] = max(maxci.get((k, ch), -1), ci)
    cap = {key: (v + 127) // 128 * 128 for key, v in cap.items()}

    # ---- shared plan: mirrored gather/scatter layout ----
    # per k: slots ordered (ch, ci, j); gather groups per ch (span ci),
    # scatter batches per (ch, ci) segment.
    plan = []
    col = 0                    # column cursor (16-entry units), shared layout
    segs = {}                  # (k, ch, ci) -> col
    max_slots = 0
    for k in range(K):
        gathers = []
        scatters = []
        slot0 = {}
        slot = 0
        for ch in range(nchk):
            if (k, ch) not in maxci:
                continue
            gcol = col
            nch_slots = 0
            for ci in range(maxci[(k, ch)] + 1):
                G = cap.get((k, ch, ci), 0)
                if G == 0:
                    continue
                segs[(k, ch, ci)] = col
                scatters.append((slot + nch_slots,
                                 slot + nch_slots + G // 128, col * 16 // 16))
                col += G // 16
                nch_slots += G // 128
            g = (ch, nch_slots, gcol)
            gathers.append(g)
            slot0[g] = slot
            slot += nch_slots
        max_slots = max(max_slots, slot)
        plan.append({"gathers": gathers, "scatters": scatters,
                     "slot0": slot0, "nslots": slot})
    ncols_i = col

    # ---- fixed plane regions ----
    w16 = np.concatenate([Wk[k].astype(F16) for k in range(K)], axis=1)
    w_plane = w16.view(np.int16).reshape(16, -1)       # [C,K*C] -> [16, cols]
    w_cols = w_plane.shape[1]
    ident_plane = np.eye(128, dtype=F16).view(np.int16).reshape(16, 1024)
    f16s = feats.astype(F16)
    f_cols = rows_pc * C // 16

    # ---- per-core planes ----
    in_maps = []
    for c in range(NCORES):
        gplane = np.zeros((16, ncols_i), np.int16)
        splane = np.zeros((16, ncols_i), np.int16)
        for (k, ch, ci), col0 in segs.items():
            G = cap[(k, ch, ci)]
            sel = sel_all.get((c, k, ch, ci), np.zeros(0, np.int64))
            n = len(sel)
            gi = np.zeros(G, np.int16)
            pad_pos = np.arange(n, G)
            si = np.empty(G, np.int16)
            si[n:] = (rows_pc + pad_pos % 1024).astype(np.int16)
            gi[:n] = (in_idx[k][sel] - ch * CHK).astype(np.int16)
            si[:n] = (out_idx[k][sel] - c * rows_pc).astype(np.int16)
            gplane[:, col0:col0 + G // 16] = _wrap16(gi)
            splane[:, col0:col0 + G // 16] = _wrap16(si)
        fsh = f16s[c * rows_pc:(c + 1) * rows_pc].reshape(16, f_cols)
        fsh = fsh.view(np.int16)
        plane = np.concatenate(
            [gplane, splane, ident_plane, w_plane, fsh], axis=1)
        in_maps.append({"plane": np.ascontiguousarray(plane)})

    return {"plan": plan, "in_maps": in_maps, "max_slots": max_slots,
            "ncols_i": ncols_i, "w_cols": w_cols, "f_cols": f_cols,
            "N": N, "C": C, "K": K, "nchk": nchk, "rows_pc": rows_pc,
            "segs": segs, "cap": cap}


def kernel(feats, kernel, in_idx, out_idx):
    feats = np.asarray(feats, np.float32)
    Wk = np.asarray(kernel, np.float32)
    in_idx = np.asarray(in_idx, np.int64)
    out_idx = np.asarray(out_idx, np.int64)

    N, C = feats.shape
    pr = _prepare(feats, Wk, in_idx, out_idx)
    K, nchk, rows_pc = pr["K"], pr["nchk"], pr["rows_pc"]
    in_maps = pr["in_maps"]

    nc = _build_program(K, C, N, nchk, pr["plan"], pr["max_slots"],
                        pr["ncols_i"], pr["w_cols"], pr["f_cols"])
    global LAST_NC, LAST_IN_MAPS
    LAST_NC = nc
    LAST_IN_MAPS = in_maps

    if BUILD_ONLY:
        return np.zeros((N, C), np.float32)

    if SIM:
        from concourse import bass_interp
        sim = bass_interp.MultiCoreSim(nc, NCORES)
        for c in range(NCORES):
            for name, arr in in_maps[c].items():
                sim.cores[c].tensor(name)[:] = arr
            sim.cores[c].tensor("out")[:] = 0
        sim.simulate()
        outs = [np.array(sim.cores[c].tensor("out")) for c in range(NCORES)]
    else:
        res = run_bass_kernel_spmd(nc, in_maps, list(range(NCORES)))
        global LAST_RESULTS
        LAST_RESULTS = res
        outs = [res.results[c]["out"] for c in range(NCORES)]

    out = np.concatenate(outs, axis=0)[:N]
    return np.ascontiguousarray(out, dtype=np.float32)
